# revision 1
# baseline (speedup 1.0000x reference)
"""Trainium2 Bass kernel for disparity cost-volume construction.

Reference computation (B=2, C=32, H=80, W=240, D=64):
    out[:, :C,  d, :, w] = x[:, :, :, w]      if w >= d else 0
    out[:, C:,  d, :, w] = y[:, :, :, w - d]  if w >= d else 0
    out shape [B, 2C, D, H, W]

Strategy: shard H across 8 cores (10 rows each; the disparity shift only
touches W so no halo). The problem is pure memory: ~78.6 MB of output
per core vs ~1.2 MB of input. Per core, load the x/y shards into SBUF
once, materialize the masked/shifted rows (zeros included) into batch
tiles of 8 disparities, and stream 16 x 4.9 MB DMAs to DRAM with 38.4 KB
contiguous descriptors per partition — SDMA line rate.

Engine discipline (HW-measured): a [128, 1200] f32 SBUF copy is ~0.7 us
on Vector and ~1.3 us on Scalar, but ~4.5 us on GpSimd — and concurrent
GpSimd SBUF traffic degrades Vector to the same ~4.5 us (port sharing).
So: left copies + all memsets on Vector, right copies on Scalar, GpSimd
untouched, and every DMA on the idle SP sequencer's HWDGE ring so the
single FIFO queue streams back-to-back with forward-only dependencies.

Layout: on-chip partition index p = (b, c, hb) with hb splitting the 10
local rows into 2 groups of 5 — full 128-partition DMA/compute width.
Per-core DRAM outputs use the custom layout [128, D, 1200] (1200 = 5
rows * 240 w); the host reassembles the canonical [B, 2C, D, H, W] view.
"""

from contextlib import ExitStack

import numpy as np

B, C, H, W, D = 2, 32, 80, 240, 64
NCORES = 8
HL = H // NCORES  # local rows per core (10)
HB, H5 = 2, 5  # local rows split: 2 partition groups x 5 rows
P = B * C * HB  # 128 partitions
F = H5 * W  # 1200 free elements per (partition, d)
ND = 4  # disparities per batch DMA
NB = D // ND  # 8 batches

_CACHE: dict = {}


def _build():
    if "nc" in _CACHE:
        return _CACHE["nc"]

    import concourse.bacc as bacc
    import concourse.mybir as mybir
    import concourse.tile as tile

    f32 = mybir.dt.float32
    nc = bacc.Bacc("TRN2", target_bir_lowering=False, debug=False)

    x_t = nc.dram_tensor("x", [P, F], f32, kind="ExternalInput")
    y_t = nc.dram_tensor("y", [P, F], f32, kind="ExternalInput")
    ol_t = nc.dram_tensor("out_l", [P, D, F], f32, kind="ExternalOutput")
    or_t = nc.dram_tensor("out_r", [P, D, F], f32, kind="ExternalOutput")

    with tile.TileContext(nc) as tc, ExitStack() as ctx:
        inpool = ctx.enter_context(tc.tile_pool(name="inp", bufs=1))
        lpool = ctx.enter_context(tc.tile_pool(name="lt", bufs=3))
        rpool = ctx.enter_context(tc.tile_pool(name="rt", bufs=3))

        x_sb = inpool.tile([P, F], f32)
        y_sb = inpool.tile([P, F], f32)
        nc.sync.dma_start(x_sb, x_t.ap())
        nc.sync.dma_start(y_sb, y_t.ap())
        xv = x_sb.rearrange("p (h w) -> p h w", h=H5)
        yv = y_sb.rearrange("p (h w) -> p h w", h=H5)

        for b in range(NB):
            db = b * ND
            lt = lpool.tile([P, ND * F], f32)
            rt = rpool.tile([P, ND * F], f32)
            ltv = lt.rearrange("p (j h w) -> p j h w", j=ND, h=H5)
            rtv = rt.rearrange("p (j h w) -> p j h w", j=ND, h=H5)
            for j in range(ND):
                d = db + j
                if d > 0:
                    nc.vector.memset(ltv[:, j, :, 0:d], 0.0)
                    nc.vector.memset(rtv[:, j, :, 0:d], 0.0)
                nc.vector.tensor_copy(ltv[:, j, :, d:W], xv[:, :, d:W])
                nc.scalar.copy(rtv[:, j, :, d:W], yv[:, :, 0 : W - d])
            nc.sync.dma_start(ol_t.ap()[:, db : db + ND, :], lt)
            nc.sync.dma_start(or_t.ap()[:, db : db + ND, :], rt)

    nc.compile()
    _CACHE["nc"] = nc
    return nc


def _shard_inputs(x: np.ndarray, y: np.ndarray):
    x = np.asarray(x, dtype=np.float32)
    y = np.asarray(y, dtype=np.float32)
    in_maps = []
    for k in range(NCORES):
        xs = np.ascontiguousarray(x[:, :, k * HL : (k + 1) * HL, :]).reshape(P, F)
        ys = np.ascontiguousarray(y[:, :, k * HL : (k + 1) * HL, :]).reshape(P, F)
        in_maps.append({"x": xs, "y": ys})
    return in_maps


def _gather(results) -> np.ndarray:
    full = np.empty((B, 2 * C, D, H, W), dtype=np.float32)
    for k in range(NCORES):
        h0 = k * HL
        for name, c0 in (("out_l", 0), ("out_r", C)):
            shard = (
                results[k][name]
                .reshape(B, C, HB, D, H5, W)
                .transpose(0, 1, 3, 2, 4, 5)
                .reshape(B, C, D, HL, W)
            )
            full[:, c0 : c0 + C, :, h0 : h0 + HL, :] = shard
    return full


def _run(x: np.ndarray, y: np.ndarray, trace: bool = False):
    from concourse.bass_utils import run_bass_kernel_spmd

    nc = _build()
    in_maps = _shard_inputs(x, y)
    res = run_bass_kernel_spmd(
        nc, in_maps, core_ids=list(range(NCORES)), trace=trace
    )
    return _gather(res.results), res


def kernel(x: np.ndarray, y: np.ndarray) -> np.ndarray:
    out, _ = _run(x, y, trace=False)
    return out



# revision 2
# speedup vs baseline: 1.5102x; 1.5102x over previous
"""Trainium2 Bass kernel for disparity cost-volume construction — v2.

Same layout/strategy as kernel.py (H sharded over 8 cores, partition
index p=(b,c,hb), per-core DRAM outputs [128, D, 1200]); changes vs
the 244-262us baseline, from the core-0 trace (DMA stream runs gap-free
at 347 GB/s but starts 16.4us in; per-NC HBM cap ~358):

1. Input x loads on the SP HWDGE ring, y on the ACT ring — parallel,
   ~2.4us instead of ~4.8us serial.
2. d=0 needs no compute (out_l[:,0]=x, out_r[:,0]=y): DMA straight
   from the input SBUF buffers the moment they land — output streaming
   starts ~5us earlier, hiding the first-batch compute gap.
3. Ramped batch sizes (1,2,4 then 8s): the first staged tile takes
   ~1us of compute instead of ~3.4us, closing the pre-stream bubble.
4. Left-tile DMAs on the SP ring, right-tile DMAs on the ACT ring:
   two independent HWDGE FIFOs so the 15 fast SDMA engines aren't
   paced by the known-slow engine 15 through a single ring's
   descriptor flow.

SBUF/partition: 2 pools x bufs=2 x 38.4 KB + 9.6 KB inputs = 163 KB.
"""

from contextlib import ExitStack

import numpy as np

B, C, H, W, D = 2, 32, 80, 240, 64
NCORES = 8
HL = H // NCORES  # local rows per core (10)
HB, H5 = 2, 5  # local rows split: 2 partition groups x 5 rows
P = B * C * HB  # 128 partitions
F = H5 * W  # 1200 free elements per (partition, d)

# disparity batches: d=0 direct from input SBUF, then ramp 1,2,4, then 8s
BATCHES = []
_d = 1
for nd in (1, 2, 4, 8, 8, 8, 8, 8, 8, 8):
    BATCHES.append((_d, nd))
    _d += nd
assert _d == D, _d

_CACHE: dict = {}


def _build():
    if "nc" in _CACHE:
        return _CACHE["nc"]

    import concourse.bacc as bacc
    import concourse.mybir as mybir
    import concourse.tile as tile

    f32 = mybir.dt.float32
    nc = bacc.Bacc("TRN2", target_bir_lowering=False, debug=False)

    x_t = nc.dram_tensor("x", [P, F], f32, kind="ExternalInput")
    y_t = nc.dram_tensor("y", [P, F], f32, kind="ExternalInput")
    ol_t = nc.dram_tensor("out_l", [P, D, F], f32, kind="ExternalOutput")
    or_t = nc.dram_tensor("out_r", [P, D, F], f32, kind="ExternalOutput")

    with tile.TileContext(nc) as tc, ExitStack() as ctx:
        inpool = ctx.enter_context(tc.tile_pool(name="inp", bufs=1))
        lpool = ctx.enter_context(tc.tile_pool(name="lt", bufs=2))
        rpool = ctx.enter_context(tc.tile_pool(name="rt", bufs=2))

        x_sb = inpool.tile([P, F], f32)
        y_sb = inpool.tile([P, F], f32)
        nc.sync.dma_start(x_sb, x_t.ap())
        nc.scalar.dma_start(y_sb, y_t.ap())
        xv = x_sb.rearrange("p (h w) -> p h w", h=H5)
        yv = y_sb.rearrange("p (h w) -> p h w", h=H5)

        # d=0: left is x verbatim, right is y verbatim — no staging
        nc.sync.dma_start(ol_t.ap()[:, 0:1, :], x_sb)
        nc.scalar.dma_start(or_t.ap()[:, 0:1, :], y_sb)

        for db, nd in BATCHES:
            lt = lpool.tile([P, nd * F], f32, tag="lt")
            rt = rpool.tile([P, nd * F], f32, tag="rt")
            ltv = lt.rearrange("p (j h w) -> p j h w", j=nd, h=H5)
            rtv = rt.rearrange("p (j h w) -> p j h w", j=nd, h=H5)
            for j in range(nd):
                d = db + j
                nc.vector.memset(ltv[:, j, :, 0:d], 0.0)
                nc.vector.memset(rtv[:, j, :, 0:d], 0.0)
                nc.vector.tensor_copy(ltv[:, j, :, d:W], xv[:, :, d:W])
                nc.scalar.copy(rtv[:, j, :, d:W], yv[:, :, 0 : W - d])
            nc.sync.dma_start(ol_t.ap()[:, db : db + nd, :], lt)
            nc.scalar.dma_start(or_t.ap()[:, db : db + nd, :], rt)

    nc.compile()
    _CACHE["nc"] = nc
    return nc


def _shard_inputs(x: np.ndarray, y: np.ndarray):
    x = np.asarray(x, dtype=np.float32)
    y = np.asarray(y, dtype=np.float32)
    in_maps = []
    for k in range(NCORES):
        xs = np.ascontiguousarray(x[:, :, k * HL : (k + 1) * HL, :]).reshape(P, F)
        ys = np.ascontiguousarray(y[:, :, k * HL : (k + 1) * HL, :]).reshape(P, F)
        in_maps.append({"x": xs, "y": ys})
    return in_maps


def _gather(results) -> np.ndarray:
    full = np.empty((B, 2 * C, D, H, W), dtype=np.float32)
    for k in range(NCORES):
        h0 = k * HL
        for name, c0 in (("out_l", 0), ("out_r", C)):
            shard = (
                results[k][name]
                .reshape(B, C, HB, D, H5, W)
                .transpose(0, 1, 3, 2, 4, 5)
                .reshape(B, C, D, HL, W)
            )
            full[:, c0 : c0 + C, :, h0 : h0 + HL, :] = shard
    return full


def _run(x: np.ndarray, y: np.ndarray, trace: bool = False):
    from concourse.bass_utils import run_bass_kernel_spmd

    nc = _build()
    in_maps = _shard_inputs(x, y)
    res = run_bass_kernel_spmd(
        nc, in_maps, core_ids=list(range(NCORES)), trace=trace
    )
    return _gather(res.results), res


def kernel(x: np.ndarray, y: np.ndarray) -> np.ndarray:
    out, _ = _run(x, y, trace=False)
    return out
